# revision 1
# baseline (speedup 1.0000x reference)
"""MetaLA decoder layer on 8 trn2 NeuronCores (Bass/Tile SPMD kernel).

Sharding: core = (batch b, sequence half). Each core fully computes its 1024
"own" tokens and recomputes the GLA state over the 1024 preceding "context"
tokens (zeros for the first half) — no collectives.

Layout strategy:
  - LayerNorms in natural [token-partition, feature] form (bn_stats + ACT
    affine with per-partition scale/bias).
  - Everything matmul-shaped in transposed [feature-partition, token] form;
    PE transposes bridge. Big GEMMs + GLA chunk matmuls in bf16 with fp32
    PSUM accumulation; gating/softmax-free scan math in fp32.
  - GLA: chunked parallel form, C=128, heads packed 4-per-128-partitions.
  - ACT tables: only {exp, ln} (set 6) + {silu} (set 18) are used; sqrt and
    sigmoid are synthesized from exp/ln to avoid table thrash.

Self-contained: hardcodes all shapes; host side shards/reassembles.
"""

import os
import numpy as np
import ml_dtypes

STAGE = int(os.environ.get("K_STAGE", "4"))
SKIP_AUG = os.environ.get("K_SKIP_AUG", "0") == "1"
SKIP_TAIL = os.environ.get("K_SKIP_TAIL", "0") == "1"
SKIP_O = os.environ.get("K_SKIP_O", "0") == "1"
PT_ONLY = os.environ.get("K_PT_ONLY", "0") == "1"
OWN_LVL = int(os.environ.get("K_OWN_LVL", "9"))

import concourse.bass as bass
import concourse.mybir as mybir
import concourse.tile as tile
from concourse import bacc
from concourse.bass_utils import run_bass_kernel_spmd
from concourse.masks import make_identity, make_upper_triangular

F32 = mybir.dt.float32
BF16 = mybir.dt.bfloat16
AF = mybir.ActivationFunctionType
OP = mybir.AluOpType

P = 128
L, B, D = 2048, 4, 1024
H = 16
DK = 512
GLU_D = 2816
NORM = 16.0
EPS = 1e-5
KD = D // P          # 8 d_in tiles
MQ = DK // P         # 4 DK tiles (= head groups)
NG = 4               # head groups (4 heads each)
CC = 128             # chunk tokens
SBT = 512            # superblock tokens
CPS = SBT // CC      # 4 chunks per superblock
NSB = 4              # 2 ctx + 2 own superblocks
MG = GLU_D // P      # 22 glu tiles
NTOK = 2048          # ctx + own tokens per core
TOWN = 1024          # own tokens per core


def _r(x):
    return np.ascontiguousarray(x)


def _bf(x):
    return np.ascontiguousarray(x.astype(ml_dtypes.bfloat16))


def prep_host_inputs(inputs):
    """Fold affines, cast/retile weights. Returns (shared weight map, per-core fn)."""
    f32 = lambda k: np.asarray(inputs[k], np.float32)
    tn_w, tn_b = f32("tn_w"), f32("tn_b")
    cn_w, cn_b = f32("cn_w"), f32("cn_b")
    conv = f32("conv_w")
    convw = conv * tn_w[:, None]                       # [1024, 4]
    convb = tn_b * conv.sum(1)                         # [1024]
    l1 = f32("l1_w") * cn_w[:, None]
    l2 = f32("l2_w") * cn_w[:, None]
    b1 = cn_b @ f32("l1_w")
    b2 = cn_b @ f32("l2_w")

    def tile_kxm(w, m_tiles):  # [D, M] -> [128, kd, m_tiles, 128]
        d_in, m = w.shape
        kd = d_in // P
        return w.reshape(kd, P, m_tiles, m // m_tiles).transpose(1, 0, 2, 3)

    shared = {
        "wq": _bf(tile_kxm(f32("q_w"), MQ).reshape(P, -1)),
        "wkg": _bf(tile_kxm(f32("kg_w"), MQ).reshape(P, -1)),
        "wv": _bf(f32("v_w").reshape(KD, P, D).transpose(1, 0, 2).reshape(P, -1)),
        "wg": _bf(f32("g_w").reshape(KD, P, D).transpose(1, 0, 2).reshape(P, -1)),
        "wout": _bf(f32("out_w").reshape(KD, P, D).transpose(1, 0, 2).reshape(P, -1)),
        # l1/l2 as [128, m(22), k(8), 128]
        "wl1": _bf(l1.reshape(KD, P, MG, P).transpose(1, 2, 0, 3).reshape(P, -1)),
        "wl2": _bf(l2.reshape(KD, P, MG, P).transpose(1, 2, 0, 3).reshape(P, -1)),
        # l3 as [128, k(22), 1024]
        "wl3": _bf(f32("l3_w").reshape(MG, P, D).transpose(1, 0, 2).reshape(P, -1)),
        "convw": _r(convw.reshape(KD, P, 4).transpose(1, 0, 2).reshape(P, -1)),
        "convb": _r(convb.reshape(KD, P).T),
        "aug": _r(f32("aug_balance").reshape(MQ, P).T),
        "b1": _r(b1.reshape(MG, P).T),
        "b2": _r(b2.reshape(MG, P).T),
    }
    x = np.asarray(inputs["x"], np.float32)

    def per_core(core):
        b, half = core // 2, core % 2
        if half == 0:
            x_seq = np.concatenate(
                [np.zeros((TOWN, D), np.float32), x[0:TOWN, b]], axis=0)
        else:
            x_seq = _r(x[:, b, :])
        flag = np.full((P, 1), float(half), np.float32)
        return {"x_seq": x_seq, "flag": flag, **shared}

    return per_core


def build_nc():
    nc = bacc.Bacc("TRN2", target_bir_lowering=False, debug=False)
    dt_in = {
        "x_seq": ([NTOK, D], F32),
        "flag": ([P, 1], F32),
        "wq": ([P, KD * MQ * P], BF16),
        "wkg": ([P, KD * MQ * P], BF16),
        "wv": ([P, KD * D], BF16),
        "wg": ([P, KD * D], BF16),
        "wout": ([P, KD * D], BF16),
        "wl1": ([P, MG * KD * P], BF16),
        "wl2": ([P, MG * KD * P], BF16),
        "wl3": ([P, MG * D], BF16),
        "convw": ([P, KD * 4], F32),
        "convb": ([P, KD], F32),
        "aug": ([P, MQ], F32),
        "b1": ([P, MG], F32),
        "b2": ([P, MG], F32),
    }
    dr = {k: nc.dram_tensor(k, shp, dt, kind="ExternalInput")
          for k, (shp, dt) in dt_in.items()}
    out_d = nc.dram_tensor("out", [TOWN, D], F32, kind="ExternalOutput")
    x1_d = nc.dram_tensor("x1buf", [TOWN, D], F32)
    z_d = nc.dram_tensor("zbuf", [P, KD * TOWN], BF16)
    glu_d = nc.dram_tensor("glubuf", [P, MG * TOWN], BF16)

    wq_v = dr["wq"].ap().rearrange("p (k m c) -> p k m c", k=KD, m=MQ)
    wkg_v = dr["wkg"].ap().rearrange("p (k m c) -> p k m c", k=KD, m=MQ)
    wv_v = dr["wv"].ap().rearrange("p (k n) -> p k n", k=KD)
    wg_v = dr["wg"].ap().rearrange("p (k n) -> p k n", k=KD)
    wout_v = dr["wout"].ap().rearrange("p (k n) -> p k n", k=KD)
    wl1_v = dr["wl1"].ap().rearrange("p (m k c) -> p m k c", m=MG, k=KD)
    wl2_v = dr["wl2"].ap().rearrange("p (m k c) -> p m k c", m=MG, k=KD)
    wl3_v = dr["wl3"].ap().rearrange("p (k n) -> p k n", k=MG)
    z_v = z_d.ap().rearrange("p (k t) -> p k t", k=KD)
    glu_v = glu_d.ap().rearrange("p (m t) -> p m t", m=MG)

    with tile.TileContext(nc) as tc:
        _emit(nc, tc, dr, out_d, x1_d, z_v, glu_v,
              wq_v, wkg_v, wv_v, wg_v, wout_v, wl1_v, wl2_v, wl3_v)
    nc.compile()
    return nc


def _stub_out(nc, tc, out_d):
    with tc.tile_pool(name="stub", bufs=1) as sp:
        zt = sp.tile([P, D], F32)
        nc.vector.memset(zt, 0.0)
        for cc in range(TOWN // P):
            nc.sync.dma_start(out=out_d.ap()[cc * P:(cc + 1) * P, :], in_=zt)


def _emit(nc, tc, dr, out_d, x1_d, z_v, glu_v,
          wq_v, wkg_v, wv_v, wg_v, wout_v, wl1_v, wl2_v, wl3_v):
    import contextlib
    ctx = contextlib.ExitStack()
    with ctx:
        sing = ctx.enter_context(tc.tile_pool(name="sing", bufs=1))
        # ---- resident weights / constants ----
        wq_sb = sing.tile([P, KD, MQ, P], BF16)
        nc.sync.dma_start(out=wq_sb, in_=wq_v)
        wkg_sb = sing.tile([P, KD, MQ, P], BF16)
        nc.sync.dma_start(out=wkg_sb, in_=wkg_v)
        wv_sb = sing.tile([P, KD, D], BF16)
        nc.sync.dma_start(out=wv_sb, in_=wv_v)
        wg_sb = sing.tile([P, KD, D], BF16)
        nc.sync.dma_start(out=wg_sb, in_=wg_v)
        wout_sb = sing.tile([P, KD, D], BF16)
        nc.sync.dma_start(out=wout_sb, in_=wout_v)
        convw_sb = sing.tile([P, KD, 4], F32)
        nc.sync.dma_start(out=convw_sb, in_=dr["convw"].ap().rearrange("p (k i) -> p k i", k=KD))
        convb_sb = sing.tile([P, KD], F32)
        nc.sync.dma_start(out=convb_sb, in_=dr["convb"].ap())
        aug_sb = sing.tile([P, MQ], F32)
        nc.sync.dma_start(out=aug_sb, in_=dr["aug"].ap())
        b1_sb = sing.tile([P, MG], F32)
        nc.sync.dma_start(out=b1_sb, in_=dr["b1"].ap())
        b2_sb = sing.tile([P, MG], F32)
        nc.sync.dma_start(out=b2_sb, in_=dr["b2"].ap())
        flag_sb = sing.tile([P, 1], F32)
        nc.sync.dma_start(out=flag_sb, in_=dr["flag"].ap())

        ident = sing.tile([P, P], F32)
        make_identity(nc, ident)
        identb = sing.tile([P, P], BF16)
        make_identity(nc, identb)
        maskc = sing.tile([P, P], F32)
        make_upper_triangular(nc, maskc, val=1.0, diag=True)
        ind4 = sing.tile([P, 4], BF16)
        nc.vector.memset(ind4, 0.0)
        for j in range(4):
            nc.vector.memset(ind4[32 * j:32 * j + 32, j:j + 1], 1.0)
        epst = sing.tile([P, 1], F32)
        nc.vector.memset(epst, EPS)

        s_st = sing.tile([P, NG, 64], F32)      # GLA state, 4 heads per group
        nc.vector.memset(s_st, 0.0)
        hT = sing.tile([P, KD, 3 + SBT], F32)   # transposed h with 3-col halo
        nc.vector.memset(hT[:, :, 0:3], 0.0)

        x_ap = dr["x_seq"].ap()

        scan = ctx.enter_context(tc.tile_pool(name="scan", bufs=1))
        pps = ctx.enter_context(tc.tile_pool(name="pps", bufs=1, space="PSUM"))

        def rstd_chain(var_ap, pfx, n):
            """rstd = exp(-0.5*ln(var+eps)); var_ap [P, n]."""
            lnv = scan.tile([P, n], F32, name=f"{pfx}_lnv", tag=f"{pfx}_lnv", bufs=2)
            nc.scalar.activation(out=lnv, in_=var_ap, func=AF.Ln,
                                 bias=epst[:, 0:1], scale=1.0)
            rstd = scan.tile([P, n], F32, name=f"{pfx}_rstd", tag=f"{pfx}_rstd", bufs=2)
            nc.scalar.activation(out=rstd, in_=lnv, func=AF.Exp, scale=-0.5)
            return rstd

        def layernorm(x_t, pfx, out_dtype, h_bufs=2):
            """natural-layout LN of [P, 1024] -> normalized tile (pure, no affine)."""
            bnst = scan.tile([P, 2, 6], F32, name=f"{pfx}_bnst", tag=f"{pfx}_bnst", bufs=2)
            nc.vector.bn_stats(out=bnst[:, 0, :], in_=x_t[:, 0:512])
            nc.vector.bn_stats(out=bnst[:, 1, :], in_=x_t[:, 512:1024])
            mv = scan.tile([P, 2], F32, name=f"{pfx}_mv", tag=f"{pfx}_mv", bufs=2)
            nc.vector.bn_aggr(out=mv, in_=bnst)
            rstd = rstd_chain(mv[:, 1:2], pfx, 1)
            nmrs = scan.tile([P, 1], F32, name=f"{pfx}_nmrs", tag=f"{pfx}_nmrs", bufs=2)
            nc.vector.scalar_tensor_tensor(out=nmrs, in0=mv[:, 0:1], scalar=-1.0,
                                           in1=rstd, op0=OP.mult, op1=OP.mult)
            h = scan.tile([P, D], out_dtype, name=f"{pfx}_h", tag=f"{pfx}_h", bufs=h_bufs)
            nc.scalar.activation(out=h, in_=x_t, func=AF.Identity,
                                 bias=nmrs[:, 0:1], scale=rstd[:, 0:1])
            return h

        # ---------------- superblock loop ----------------
        for sb in range(NSB if STAGE >= 2 else 2):
            own = sb >= 2
            tok0 = sb * SBT

            if sb > 0:
                halo = scan.tile([P, KD, 3], F32, tag="halo", bufs=2)
                nc.vector.tensor_copy(out=halo, in_=hT[:, :, SBT:SBT + 3])
                nc.vector.tensor_copy(out=hT[:, :, 0:3], in_=halo)

            for cc in range(CPS):
                x_t = scan.tile([P, D], F32, tag="x_nat", bufs=2,
                                name=f"x_nat_{sb}_{cc}")
                nc.sync.dma_start(out=x_t, in_=x_ap[tok0 + cc * CC: tok0 + (cc + 1) * CC, :])
                h = layernorm(x_t, "ln1", F32)
                # transpose h into hT
                for half in range(2):
                    ptr = pps.tile([P, SBT], F32, tag="pstr", bufs=2,
                                   name=f"htr_{sb}_{cc}_{half}")
                    for j in range(4):
                        dt_i = half * 4 + j
                        nc.tensor.transpose(ptr[:, j * P:(j + 1) * P],
                                            h[:, dt_i * P:(dt_i + 1) * P], ident)
                    nc.scalar.copy(
                        out=hT[:, half * 4:(half + 1) * 4, 3 + cc * CC: 3 + (cc + 1) * CC],
                        in_=ptr.rearrange("p (j c) -> p j c", j=4))

            # ---- conv + silu -> yT (bf16) ----
            yT = scan.tile([P, KD, SBT], BF16, tag="yT", bufs=1, name=f"yT_{sb}")
            for dt_i in range(KD):
                a0 = scan.tile([P, SBT], F32, tag="cacc", bufs=2, name=f"ca0_{sb}_{dt_i}")
                nc.vector.tensor_scalar(out=a0, in0=hT[:, dt_i, 0:SBT],
                                        scalar1=convw_sb[:, dt_i, 0:1], scalar2=None,
                                        op0=OP.mult)
                a1 = scan.tile([P, SBT], F32, tag="cacc", bufs=2, name=f"ca1_{sb}_{dt_i}")
                nc.vector.scalar_tensor_tensor(out=a1, in0=hT[:, dt_i, 1:1 + SBT],
                                               scalar=convw_sb[:, dt_i, 1:2], in1=a0,
                                               op0=OP.mult, op1=OP.add)
                a2 = scan.tile([P, SBT], F32, tag="cacc", bufs=2, name=f"ca2_{sb}_{dt_i}")
                nc.vector.scalar_tensor_tensor(out=a2, in0=hT[:, dt_i, 2:2 + SBT],
                                               scalar=convw_sb[:, dt_i, 2:3], in1=a1,
                                               op0=OP.mult, op1=OP.add)
                a3 = scan.tile([P, SBT], F32, tag="cacc", bufs=2, name=f"ca3_{sb}_{dt_i}")
                nc.vector.scalar_tensor_tensor(out=a3, in0=hT[:, dt_i, 3:3 + SBT],
                                               scalar=convw_sb[:, dt_i, 3:4], in1=a2,
                                               op0=OP.mult, op1=OP.add)
                nc.scalar.activation(out=yT[:, dt_i, :], in_=a3, func=AF.Silu,
                                     bias=convb_sb[:, dt_i:dt_i + 1], scale=1.0)

            # ---- projections (superblock granularity for q/kg) ----
            def proj_kxm(w_sb, out_tile, m, dtype, w_m=None):
                if w_m is None:
                    w_m = m
                ps = pps.tile([P, SBT], F32, tag="psmm", bufs=2,
                              name=f"pj_{sb}_{w_sb.name}_{w_m}")
                for k in range(KD):
                    nc.tensor.matmul(ps, w_sb[:, k, w_m, :], yT[:, k, :],
                                     start=(k == 0), stop=(k == KD - 1))
                nc.scalar.copy(out=out_tile[:, m, :], in_=ps)

            # ---- per-group: kg/q projections + gating (fp32, exp/ln only) ----
            kt = scan.tile([P, MQ, SBT], BF16, tag="kt", bufs=1, name=f"kt_{sb}")
            khat = scan.tile([P, MQ, SBT], BF16, tag="khat", bufs=1, name=f"khat_{sb}")
            k_ = scan.tile([P, MQ, SBT], BF16, tag="k_", bufs=1, name=f"k__{sb}")
            dCt = scan.tile([P, MQ, CPS], F32, tag="dCt", bufs=1, name=f"dCt_{sb}")
            qTb = scan.tile([P, MQ, SBT], BF16, tag="qTb", bufs=1, name=f"qTb_{sb}")
            if own:
                qt = scan.tile([P, MQ, SBT], BF16, tag="qt", bufs=1, name=f"qt_{sb}")
            for g in range(NG):
                kgg = scan.tile([P, SBT], F32, tag="kgg", bufs=2, name=f"kgg_{sb}_{g}")
                proj_kxm(wkg_sb, kgg[:, None, :], 0, F32, w_m=g)
                if own:
                    proj_kxm(wq_sb, qTb[:, g:g + 1, :], 0, BF16, w_m=g)
                t0 = scan.tile([P, SBT], F32, tag="gsc", bufs=4, name=f"g0_{sb}_{g}")
                nc.scalar.activation(out=t0, in_=kgg, func=AF.Exp, scale=-1.0)
                sp = scan.tile([P, SBT], F32, tag="gsc", bufs=4, name=f"g1_{sb}_{g}")
                nc.scalar.activation(out=sp, in_=t0, func=AF.Ln, bias=1.0, scale=1.0)
                A = scan.tile([P, SBT], F32, tag="gsc", bufs=4, name=f"g2_{sb}_{g}")
                for cc in range(CPS):
                    nc.vector.tensor_tensor_scan(
                        out=A[:, cc * CC:(cc + 1) * CC],
                        data0=sp[:, cc * CC:(cc + 1) * CC],
                        data1=sp[:, cc * CC:(cc + 1) * CC],
                        initial=0.0, op0=OP.add, op1=OP.bypass)
                ek = scan.tile([P, SBT], F32, tag="gsc", bufs=4, name=f"g3_{sb}_{g}")
                nc.scalar.activation(out=ek, in_=sp, func=AF.Exp, scale=-1.0 / NORM)
                nc.vector.tensor_scalar(out=k_[:, g, :], in0=ek, scalar1=-1.0,
                                        scalar2=1.0, op0=OP.mult, op1=OP.add)
                nc.scalar.activation(out=dCt[:, g, :],
                                     in_=A.rearrange("p (cc c) -> p cc c", cc=CPS)[:, :, CC - 1],
                                     func=AF.Exp, scale=-1.0 / NORM)
                if own:
                    eA = scan.tile([P, SBT], F32, tag="gsc", bufs=4, name=f"g4_{sb}_{g}")
                    nc.scalar.activation(out=eA, in_=A, func=AF.Exp, scale=-1.0 / NORM)
                    nc.vector.tensor_tensor(out=qt[:, g, :], in0=qTb[:, g, :], in1=eA,
                                            op=OP.mult)
                erA = scan.tile([P, SBT], F32, tag="gsc", bufs=4, name=f"g5_{sb}_{g}")
                nc.scalar.activation(out=erA, in_=A, func=AF.Exp, scale=1.0 / NORM)
                nc.vector.tensor_tensor(out=kt[:, g, :], in0=k_[:, g, :], in1=erA,
                                        op=OP.mult)
                nc.vector.tensor_tensor(
                    out=khat[:, g, :].rearrange("p (cc c) -> p cc c", cc=CPS),
                    in0=kt[:, g, :].rearrange("p (cc c) -> p cc c", cc=CPS),
                    in1=dCt[:, g, :, None].broadcast_to((P, CPS, CC)),
                    op=OP.mult)

            if sb == 2:
                nc.vector.tensor_scalar(out=s_st.rearrange("p g c -> p (g c)"),
                                        in0=s_st.rearrange("p g c -> p (g c)"),
                                        scalar1=flag_sb[:, 0:1], scalar2=None,
                                        op0=OP.mult)

            # ---- chunk scan ----
            for cc in range(CPS):
                csl = slice(cc * CC, (cc + 1) * CC)
                # v (and g) projections for this chunk
                v_t = scan.tile([P, D], BF16, tag="v_nat", bufs=3,
                                name=f"v_{sb}_{cc}")
                for n in range(2):
                    ps = pps.tile([P, 512], F32, tag="psmm", bufs=2,
                                  name=f"pv_{sb}_{cc}_{n}")
                    for k in range(KD):
                        nc.tensor.matmul(ps, yT[:, k, cc * CC:(cc + 1) * CC],
                                         wv_sb[:, k, n * 512:(n + 1) * 512],
                                         start=(k == 0), stop=(k == KD - 1))
                    nc.scalar.copy(out=v_t[:, n * 512:(n + 1) * 512], in_=ps)
                if own:
                    g_t = scan.tile([P, D], BF16, tag="g_silu", bufs=2,
                                    name=f"gs_{sb}_{cc}")
                    for n in range(2):
                        ps = pps.tile([P, 512], F32, tag="psmm", bufs=2,
                                      name=f"pg_{sb}_{cc}_{n}")
                        for k in range(KD):
                            nc.tensor.matmul(ps, yT[:, k, cc * CC:(cc + 1) * CC],
                                             wg_sb[:, k, n * 512:(n + 1) * 512],
                                             start=(k == 0), stop=(k == KD - 1))
                        nc.scalar.activation(out=g_t[:, n * 512:(n + 1) * 512], in_=ps,
                                             func=AF.Silu)
                # k_hat transpose -> natural (per group)
                knat = []
                for g in range(NG):
                    kn = scan.tile([P, P], BF16, tag="knat", bufs=6,
                                   name=f"knat_{sb}_{cc}_{g}")
                    nc.sync.dma_start(out=kn, in_=khat[:, g, csl], transpose=True)
                    knat.append(kn)

                if own and SKIP_O:
                    o_sb = scan.tile([P, D], F32, tag="o_sb", bufs=1,
                                     name=f"osb_{sb}_{cc}")
                    nc.vector.memset(o_sb, 0.5)
                    if SKIP_TAIL:
                        trow0 = tok0 + cc * CC - TOWN
                        nc.sync.dma_start(out=x1_d.ap()[trow0:trow0 + CC, :], in_=o_sb)
                elif own:
                    s_bf = scan.tile([P, H, 64], BF16, tag="s_bf", bufs=1,
                                     name=f"sbf_{sb}_{cc}")
                    nc.vector.memset(s_bf, 0.0)
                    for hh in range(4):
                        hs = slice(32 * hh, 32 * hh + 32)
                        nc.vector.tensor_copy(
                            out=s_bf[hs, :, :].rearrange(
                                "p (g x) c -> p g x c", x=4)[:, :, hh, :],
                            in_=s_st[hs, :, :])
                    o_ps = pps.tile([P, D], F32, tag="pso", bufs=1,
                                    name=f"ops_{sb}_{cc}")
                    ptsb = []
                    for g in range(NG):
                        pt = scan.tile([P, SBT], BF16, tag="ptsb", bufs=1,
                                       name=f"pt_{sb}_{cc}_{g}")
                        for hh in range(4):
                            hs = slice(32 * hh, 32 * hh + 32)
                            pps_t = pps.tile([P, P], F32, tag="psmm", bufs=2,
                                             name=f"ptps_{sb}_{cc}_{g}_{hh}")
                            nc.tensor.matmul(pps_t, kt[hs, g, csl], qt[hs, g, csl],
                                             start=True, stop=True,
                                             tile_position=(32 * hh, 0))
                            nc.vector.tensor_tensor(out=pt[:, hh * P:(hh + 1) * P],
                                                    in0=pps_t, in1=maskc, op=OP.mult)
                        ptsb.append(pt)
                    for g in range(NG):
                        for hh in range(4):
                            hd = 4 * g + hh
                            hs = slice(32 * hh, 32 * hh + 32)
                            osl = slice(64 * hd, 64 * hd + 64)
                            nc.tensor.matmul(o_ps[:, osl], qt[:, g, csl],
                                             s_bf[:, hd, :], start=True, stop=False)
                            nc.tensor.matmul(o_ps[:, osl],
                                             ptsb[g][:, hh * P:(hh + 1) * P],
                                             v_t[:, osl], start=False, stop=True)
                    # aug term
                    if SKIP_AUG:
                        o_sb = scan.tile([P, D], F32, tag="o_sb", bufs=1,
                                         name=f"osb_{sb}_{cc}")
                        nc.scalar.copy(out=o_sb, in_=o_ps)
                    else:
                        aug_t = None
                    if not SKIP_AUG:
                        aug_t = scan.tile([P, 16], F32, tag="aug_t", bufs=2,
                                          name=f"augt_{sb}_{cc}")
                        for g in range(NG):
                            qk = scan.tile([P, P], BF16, tag="qk", bufs=2,
                                           name=f"qk_{sb}_{cc}_{g}")
                            nc.vector.scalar_tensor_tensor(out=qk, in0=qTb[:, g, csl],
                                                           scalar=aug_sb[:, g:g + 1],
                                                           in1=k_[:, g, csl],
                                                           op0=OP.mult, op1=OP.mult)
                            aps = pps.tile([4, P], F32, tag="psaug", bufs=2,
                                           name=f"aps_{sb}_{cc}_{g}")
                            nc.tensor.matmul(aps, ind4, qk, start=True, stop=True)
                            augg = scan.tile([4, P], F32, tag="augg", bufs=2,
                                             name=f"augg_{sb}_{cc}_{g}")
                            nc.scalar.copy(out=augg, in_=aps)
                            atps = pps.tile([P, 4], F32, tag="psaug", bufs=2,
                                            name=f"atps_{sb}_{cc}_{g}")
                            nc.tensor.transpose(atps, augg, ident[0:4, 0:4])
                            nc.vector.tensor_copy(out=aug_t[:, 4 * g:4 * g + 4], in_=atps)
                        augv = scan.tile([P, D], F32, tag="etmp", bufs=3,
                                         name=f"augv_{sb}_{cc}")
                        nc.vector.tensor_tensor(
                            out=augv.rearrange("p (h c) -> p h c", h=H),
                            in0=v_t.rearrange("p (h c) -> p h c", h=H),
                            in1=aug_t[:, :, None].broadcast_to((P, H, 64)),
                            op=OP.mult)
                        sg1 = scan.tile([P, D], F32, tag="etmp", bufs=3,
                                        name=f"sg1_{sb}_{cc}")
                        nc.scalar.activation(out=sg1, in_=augv, func=AF.Exp, scale=-1.0)
                        sg2 = scan.tile([P, D], F32, tag="etmp", bufs=3,
                                        name=f"sg2_{sb}_{cc}")
                        nc.scalar.activation(out=sg2, in_=sg1, func=AF.Ln, bias=1.0, scale=1.0)
                        sg3 = scan.tile([P, D], F32, tag="etmp", bufs=3,
                                        name=f"sg3_{sb}_{cc}")
                        nc.scalar.activation(out=sg3, in_=sg2, func=AF.Exp, scale=-1.0)
                        o_sb = scan.tile([P, D], F32, tag="o_sb", bufs=1,
                                         name=f"osb_{sb}_{cc}")
                        nc.vector.tensor_tensor(out=o_sb, in0=o_ps, in1=sg3, op=OP.add)
                    if SKIP_TAIL:
                        trow0 = tok0 + cc * CC - TOWN
                        nc.sync.dma_start(out=x1_d.ap()[trow0:trow0 + CC, :], in_=o_sb)
                        continue
                    # groupnorm over 64 per head (reduce-based)
                    o_sbh = o_sb.rearrange("p (h c) -> p h c", h=H)
                    osq = scan.tile([P, D], F32, tag="etmp", bufs=3,
                                    name=f"osq_{sb}_{cc}")
                    nc.vector.tensor_tensor(out=osq, in0=o_sb, in1=o_sb, op=OP.mult)
                    s1 = scan.tile([P, H], F32, tag="s1", bufs=2, name=f"s1_{sb}_{cc}")
                    nc.vector.tensor_reduce(out=s1, in_=o_sbh,
                                            axis=mybir.AxisListType.X, op=OP.add)
                    s2 = scan.tile([P, H], F32, tag="s2", bufs=2, name=f"s2_{sb}_{cc}")
                    nc.vector.tensor_reduce(out=s2,
                                            in_=osq.rearrange("p (h c) -> p h c", h=H),
                                            axis=mybir.AxisListType.X, op=OP.add)
                    mo = scan.tile([P, H], F32, tag="mo", bufs=2, name=f"mo_{sb}_{cc}")
                    nc.vector.tensor_scalar(out=mo, in0=s1, scalar1=1.0 / 64.0,
                                            scalar2=None, op0=OP.mult)
                    mo2 = scan.tile([P, H], F32, tag="mo2", bufs=2, name=f"mo2_{sb}_{cc}")
                    nc.vector.tensor_tensor(out=mo2, in0=mo, in1=mo, op=OP.mult)
                    varo = scan.tile([P, H], F32, tag="varo", bufs=2,
                                     name=f"varo_{sb}_{cc}")
                    nc.vector.scalar_tensor_tensor(out=varo, in0=s2, scalar=1.0 / 64.0,
                                                   in1=mo2, op0=OP.mult,
                                                   op1=OP.subtract)
                    rstdo = rstd_chain(varo, f"go", H)
                    nmrso = scan.tile([P, H], F32, tag="nmrso", bufs=2,
                                      name=f"nmo_{sb}_{cc}")
                    nc.vector.scalar_tensor_tensor(out=nmrso, in0=mo,
                                                   scalar=-1.0, in1=rstdo,
                                                   op0=OP.mult, op1=OP.mult)
                    on1 = scan.tile([P, D], F32, tag="etmp", bufs=3,
                                    name=f"on1_{sb}_{cc}")
                    nc.vector.tensor_tensor(
                        out=on1.rearrange("p (h c) -> p h c", h=H),
                        in0=o_sb.rearrange("p (h c) -> p h c", h=H),
                        in1=rstdo[:, :, None].broadcast_to((P, H, 64)), op=OP.mult)
                    on2 = scan.tile([P, D], F32, tag="etmp", bufs=3,
                                    name=f"on2_{sb}_{cc}")
                    nc.vector.tensor_tensor(
                        out=on2.rearrange("p (h c) -> p h c", h=H),
                        in0=on1.rearrange("p (h c) -> p h c", h=H),
                        in1=nmrso[:, :, None].broadcast_to((P, H, 64)), op=OP.add)
                    og = scan.tile([P, D], BF16, tag="og", bufs=2,
                                   name=f"og_{sb}_{cc}")
                    nc.vector.tensor_tensor(out=og, in0=on2, in1=g_t, op=OP.mult)
                    # transpose og -> ogT (xbar)
                    ogT = scan.tile([P, KD, P], BF16, tag="ogT", bufs=2,
                                    name=f"ogT_{sb}_{cc}")
                    for dt_i in range(KD):
                        nc.sync.dma_start(out=ogT[:, dt_i, :],
                                          in_=og[:, dt_i * P:(dt_i + 1) * P],
                                          transpose=True)
                    # out proj + residual
                    x1 = scan.tile([P, D], F32, tag="x1", bufs=2, name=f"x1_{sb}_{cc}")
                    x_res = scan.tile([P, D], F32, tag="x_res", bufs=2,
                                      name=f"xres_{sb}_{cc}")
                    nc.sync.dma_start(
                        out=x_res, in_=x_ap[tok0 + cc * CC: tok0 + (cc + 1) * CC, :])
                    for n in range(2):
                        ops2 = pps.tile([P, 512], F32, tag="psmm", bufs=2,
                                        name=f"opj_{sb}_{cc}_{n}")
                        for k in range(KD):
                            nc.tensor.matmul(ops2, ogT[:, k, :],
                                             wout_sb[:, k, n * 512:(n + 1) * 512],
                                             start=(k == 0), stop=(k == KD - 1))
                        nc.vector.scalar_tensor_tensor(
                            out=x1[:, n * 512:(n + 1) * 512], in0=ops2, scalar=1.0,
                            in1=x_res[:, n * 512:(n + 1) * 512],
                            op0=OP.bypass, op1=OP.add)
                    trow = tok0 + cc * CC - TOWN
                    nc.sync.dma_start(out=x1_d.ap()[trow:trow + CC, :], in_=x1)
                    # LN2 + z transpose -> zbuf
                    z = layernorm(x1, "ln2", BF16, h_bufs=1)
                    zTc = scan.tile([P, KD, P], BF16, tag="zTc", bufs=2,
                                    name=f"zTc_{sb}_{cc}")
                    for dt_i in range(KD):
                        nc.sync.dma_start(out=zTc[:, dt_i, :],
                                          in_=z[:, dt_i * P:(dt_i + 1) * P],
                                          transpose=True)
                    nc.sync.dma_start(out=z_v[:, :, trow:trow + CC], in_=zTc)

                # state update (ctx and own)
                for g in range(NG):
                    dsp = pps.tile([P, 64], F32, tag="pso", bufs=1,
                                   name=f"dsp_{sb}_{cc}_{g}")
                    for hh in range(4):
                        hd = 4 * g + hh
                        hs = slice(32 * hh, 32 * hh + 32)
                        nc.tensor.matmul(dsp[hs, :], knat[g][:, hs],
                                         v_t[:, 64 * hd:64 * hd + 64],
                                         start=True, stop=True,
                                         tile_position=(0, 32 * hh))
                    nc.vector.scalar_tensor_tensor(out=s_st[:, g, :],
                                                   in0=s_st[:, g, :],
                                                   scalar=dCt[:, g, cc:cc + 1],
                                                   in1=dsp, op0=OP.mult, op1=OP.add)

    # ---------------- GLU phase ----------------
    if STAGE < 3:
        _stub_out(nc, tc, out_d)
        return
    ctx2 = contextlib.ExitStack()
    with ctx2:
        gl = ctx2.enter_context(tc.tile_pool(name="gl", bufs=1))
        glps = ctx2.enter_context(tc.tile_pool(name="glps", bufs=1, space="PSUM"))
        zT = gl.tile([P, KD, TOWN], BF16)
        nc.sync.dma_start(out=zT, in_=z_v)
        for m in range(MG):
            l1m = gl.tile([P, KD, P], BF16, tag="l1m", bufs=3, name=f"l1m_{m}")
            nc.sync.dma_start(out=l1m, in_=wl1_v[:, m, :, :])
            l2m = gl.tile([P, KD, P], BF16, tag="l2m", bufs=3, name=f"l2m_{m}")
            nc.sync.dma_start(out=l2m, in_=wl2_v[:, m, :, :])
            g1s = gl.tile([P, TOWN], F32, tag="g1s", bufs=2, name=f"g1s_{m}")
            glum = gl.tile([P, TOWN], BF16, tag="glum", bufs=2, name=f"glum_{m}")
            for t in range(2):
                tsl = slice(t * 512, (t + 1) * 512)
                g1p = glps.tile([P, 512], F32, tag="psg", bufs=4, name=f"g1p_{m}_{t}")
                for k in range(KD):
                    nc.tensor.matmul(g1p, l1m[:, k, :], zT[:, k, tsl],
                                     start=(k == 0), stop=(k == KD - 1))
                nc.scalar.activation(out=g1s[:, tsl], in_=g1p, func=AF.Silu,
                                     bias=b1_sb[:, m:m + 1], scale=1.0)
                g2p = glps.tile([P, 512], F32, tag="psg", bufs=4, name=f"g2p_{m}_{t}")
                for k in range(KD):
                    nc.tensor.matmul(g2p, l2m[:, k, :], zT[:, k, tsl],
                                     start=(k == 0), stop=(k == KD - 1))
                nc.vector.scalar_tensor_tensor(out=glum[:, tsl], in0=g2p,
                                               scalar=b2_sb[:, m:m + 1],
                                               in1=g1s[:, tsl],
                                               op0=OP.add, op1=OP.mult)
            nc.sync.dma_start(out=glu_v[:, m, :], in_=glum)

    if STAGE < 4:
        _stub_out(nc, tc, out_d)
        return
    ctx3 = contextlib.ExitStack()
    with ctx3:
        g3 = ctx3.enter_context(tc.tile_pool(name="g3", bufs=1))
        g3ps = ctx3.enter_context(tc.tile_pool(name="g3ps", bufs=1, space="PSUM"))
        for t in range(2):
            gluT = g3.tile([P, MG, 512], BF16, tag="gluT", bufs=2, name=f"gluT_{t}")
            nc.sync.dma_start(out=gluT, in_=glu_v[:, :, t * 512:(t + 1) * 512])
            psf = [g3ps.tile([P, 512], F32, tag="psf", bufs=8, name=f"psf_{t}_{i}")
                   for i in range(8)]
            for k in range(MG):
                l3k = g3.tile([P, D], BF16, tag="l3k", bufs=3, name=f"l3k_{t}_{k}")
                nc.sync.dma_start(out=l3k, in_=wl3_v[:, k, :])
                for cc in range(4):
                    for n in range(2):
                        nc.tensor.matmul(psf[cc * 2 + n],
                                         gluT[:, k, cc * P:(cc + 1) * P],
                                         l3k[:, n * 512:(n + 1) * 512],
                                         start=(k == 0), stop=(k == MG - 1))
            for cc in range(4):
                trow = t * 512 + cc * CC
                x1c = g3.tile([P, D], F32, tag="x1c", bufs=2, name=f"x1c_{t}_{cc}")
                nc.sync.dma_start(out=x1c, in_=x1_d.ap()[trow:trow + CC, :])
                outc = g3.tile([P, D], F32, tag="outc", bufs=2, name=f"outc_{t}_{cc}")
                for n in range(2):
                    nc.vector.scalar_tensor_tensor(
                        out=outc[:, n * 512:(n + 1) * 512], in0=psf[cc * 2 + n],
                        scalar=1.0, in1=x1c[:, n * 512:(n + 1) * 512],
                        op0=OP.bypass, op1=OP.add)
                nc.sync.dma_start(out=out_d.ap()[trow:trow + CC, :], in_=outc)


_NC_CACHE = {}


def get_nc():
    if "nc" not in _NC_CACHE:
        _NC_CACHE["nc"] = build_nc()
    return _NC_CACHE["nc"]


def kernel(**inputs):
    nc = get_nc()
    per_core = prep_host_inputs(inputs)
    in_maps = [per_core(c) for c in range(8)]
    res = run_bass_kernel_spmd(nc, in_maps, core_ids=list(range(8)))
    out = np.zeros((L, B, D), np.float32)
    for c in range(8):
        b, half = c // 2, c % 2
        out[half * TOWN:(half + 1) * TOWN, b, :] = res.results[c]["out"]
    return out

